# revision 7
# baseline (speedup 1.0000x reference)
"""Segmented (ragged) single-query attention on 8 TRN2 NeuronCores.

Problem: B=32 batch rows, each with one query q[256], keys/values K/V[4096,256]
and 64 sorted separator positions. Segment i of row b covers key positions p
with seps[b,i] < p < seps[b,i+1]; softmax attention is computed independently
per segment. Output y[32,63,256] (+ y_mask of ones).

Sharding: data-parallel over B — each of the 8 cores handles 4 batch rows,
no cross-core communication.

Per-core algorithm (r = 0..3 rows, c = 0..31 chunks of 128 key positions):
  e[p]  = (K[p,:] . q) / 16          one fused DVE scalar_tensor_tensor per
                                     [128,256] K-tile with accum_out (row-sum)
  x     = exp(e)                     ScalarE; no max-subtraction needed:
                                     e ~ N(0,1), overflow impossible (|e|<~25)
  M^T[p,i] = (pos_p > seps[i])*(pos_p < seps[i+1])   2 DVE ops per chunk for
                                     all 4 rows at once ([128, 4*63] tiles)
  W = M^T * x                        per-row tensor_scalar
  num[63,256] += W_r.T @ V_tile      PE matmul, natural V layout, PSUM accum
  den[63,1]   += W_r.T @ ones
  y = num * 1/max(den, 1e-30)
Softmax weights are invariant to the max-shift, so skipping it is exact up to
fp rounding; empty segments give num=0, den=0 -> y=0, matching the reference.
"""

import sys

for _p in ("/opt/trn_rl_repo",):
    if _p not in sys.path:
        sys.path.insert(0, _p)

import numpy as np

import concourse.bass as bass
import concourse.tile as tile
from concourse import bacc, mybir
from concourse.bass_utils import run_bass_kernel_spmd

N_CORES = 8
B, LK, D, NSEP = 32, 4096, 256, 64
S = NSEP - 1  # 63 segments
RPC = B // N_CORES  # rows per core = 4
NCHUNK = LK // 128  # 32
F32 = mybir.dt.float32
F32R = mybir.dt.float32r
I32 = mybir.dt.int32
OP = mybir.AluOpType


def build_nc():
    nc = bacc.Bacc(
        "TRN2",
        target_bir_lowering=False,
        debug=False,
        enable_asserts=False,
        num_devices=N_CORES,
    )
    q_d = nc.dram_tensor("Q", [RPC, D], F32, kind="ExternalInput").ap()
    k_d = nc.dram_tensor("K", [RPC, LK, D], F32, kind="ExternalInput").ap()
    v_d = nc.dram_tensor("V", [RPC, LK, D], F32R, kind="ExternalInput").ap()
    # seps int64 bytes passed as int32 pairs (little-endian; values < 4096 so
    # the low word at even indices is the value).
    s_d = nc.dram_tensor("seps", [1, 2 * RPC * NSEP], I32, kind="ExternalInput").ap()
    y_d = nc.dram_tensor("y", [RPC, S, D], F32, kind="ExternalOutput").ap()

    with tile.TileContext(nc) as tc:
        build_tile_kernel(tc, q_d, k_d, v_d, s_d, y_d)

    nc.compile()
    return nc


def build_tile_kernel(tc, q_d, k_d, v_d, s_d, y_d):
    nc = tc.nc
    from contextlib import ExitStack

    with ExitStack() as ctx:
        const = ctx.enter_context(tc.tile_pool(name="const", bufs=1))
        setup_ps = ctx.enter_context(
            tc.tile_pool(name="setup_ps", bufs=1, space="PSUM")
        )

        # --- constants -------------------------------------------------
        ones_col = const.tile([128, 1], F32, tag="ones_col")
        nc.vector.memset(ones_col[:], 1.0)
        ones2r = const.tile([128, 2], F32R, tag="ones2r")
        nc.vector.tensor_copy(ones2r[:, 0:1], ones_col[:])
        nc.vector.tensor_copy(ones2r[:, 1:2], ones_col[:])
        ones_row = const.tile([1, 128], F32, tag="ones_row")
        nc.vector.memset(ones_row[:], 1.0)

        pos_i = const.tile([128, NCHUNK], I32, tag="pos_i")
        nc.gpsimd.iota(pos_i[:], pattern=[[128, NCHUNK]], base=0, channel_multiplier=1)
        pos_f = const.tile([128, NCHUNK], F32, tag="pos_f")
        nc.vector.tensor_copy(pos_f[:], pos_i[:])

        # --- seps -> lo/hi rows, broadcast across partitions -----------
        seps_raw = const.tile([1, 2 * RPC * NSEP], I32, tag="seps_raw")
        nc.sync.dma_start(seps_raw[:], s_d[:])
        seps_f = const.tile([1, RPC * NSEP], F32, tag="seps_f")
        # even int32 columns = low words = the values
        raw_pairs = seps_raw[:].rearrange("p (n two) -> p n two", two=2)
        nc.vector.tensor_copy(
            seps_f[:].rearrange("p (n one) -> p n one", one=1),
            raw_pairs[:, :, 0:1],
        )
        # packed [1, 504]: cols 0:252 = lo (seps[r, 0:63]), 252:504 = hi
        # (seps[r, 1:64]), r-major blocks of 63.
        packed = const.tile([1, 2 * RPC * S], F32, tag="packed")
        by_row = seps_f[:].rearrange("p (r i) -> p r i", i=NSEP)
        nc.vector.tensor_copy(
            packed[:, 0 : RPC * S].rearrange("p (r i) -> p r i", i=S),
            by_row[:, :, 0:S],
        )
        nc.vector.tensor_copy(
            packed[:, RPC * S : 2 * RPC * S].rearrange("p (r i) -> p r i", i=S),
            by_row[:, :, 1 : S + 1],
        )
        lohi_ps = setup_ps.tile([128, 2 * RPC * S], F32, tag="lohi_ps")
        nc.tensor.matmul(lohi_ps[:], ones_row[:], packed[:], start=True, stop=True)
        lohi = const.tile([128, 2 * RPC * S], F32, tag="lohi")
        nc.vector.tensor_copy(lohi[:], lohi_ps[:])
        lo_b = lohi[:, 0 : RPC * S]
        hi_b = lohi[:, RPC * S : 2 * RPC * S]

        # --- q broadcast to all 128 partitions, one tile per row -------
        qb = []
        for r in range(RPC):
            q_sb = const.tile([1, D], F32, tag=f"q_sb{r}")
            nc.sync.dma_start(q_sb[:], q_d[r : r + 1, :])
            q_ps = setup_ps.tile([128, D], F32, tag=f"q_ps{r % 2}")
            nc.tensor.matmul(q_ps[:], ones_row[:], q_sb[:], start=True, stop=True)
            qb_r = const.tile([128, D], F32, tag=f"qb{r}")
            nc.vector.tensor_copy(qb_r[:], q_ps[:])
            qb.append(qb_r)

        # --- per-row PSUM accumulators [63, 257] (num | den) -----------
        acc_pool = ctx.enter_context(tc.tile_pool(name="acc", bufs=1, space="PSUM"))
        acc = [acc_pool.tile([S, D + 2], F32, tag=f"acc{r}", name=f"acc{r}") for r in range(RPC)]

        kpool = ctx.enter_context(tc.tile_pool(name="kp", bufs=8))
        vpool = ctx.enter_context(tc.tile_pool(name="vp", bufs=8))
        spool = ctx.enter_context(tc.tile_pool(name="scratch", bufs=8))
        epool = ctx.enter_context(tc.tile_pool(name="ep", bufs=4))
        wpool = ctx.enter_context(tc.tile_pool(name="wp", bufs=4))

        # --- main loop -------------------------------------------------
        for c in range(NCHUNK):
            kt = []
            vt = []
            for r in range(RPC):
                k_t = kpool.tile([128, D], F32, tag=f"k{r}")
                nc.sync.dma_start(k_t[:], k_d[r, 128 * c : 128 * (c + 1), :])
                kt.append(k_t)
                v_t = vpool.tile([128, D + 2], F32R, tag=f"v{r}")
                nc.sync.dma_start(v_t[:, 0:D], v_d[r, 128 * c : 128 * (c + 1), :])
                nc.gpsimd.tensor_copy(v_t[:, D : D + 2], ones2r[:])
                vt.append(v_t)

            pos_c = pos_f[:, c : c + 1]

            e_t = epool.tile([128, RPC], F32, tag="e")
            for r in range(RPC):
                scr = spool.tile([128, D], F32, tag="scr")
                nc.vector.scalar_tensor_tensor(
                    scr[:],
                    kt[r][:],
                    1.0 / 16.0,
                    qb[r][:],
                    op0=OP.mult,
                    op1=OP.mult,
                    accum_out=e_t[:, r : r + 1],
                )
            x_t = epool.tile([128, RPC], F32, tag="x")
            nc.scalar.activation(x_t[:], e_t[:], mybir.ActivationFunctionType.Exp)

            a_t = wpool.tile([128, RPC * S], F32, tag="a")
            nc.vector.tensor_scalar(a_t[:], lo_b, pos_c, None, op0=OP.is_lt)
            w_t = wpool.tile([128, RPC * S], F32, tag="w")
            nc.vector.scalar_tensor_tensor(
                w_t[:], hi_b, pos_c, a_t[:], op0=OP.is_gt, op1=OP.mult
            )
            wx_t = wpool.tile([128, RPC * S], F32R, tag="wx")
            for r in range(RPC):
                nc.vector.tensor_scalar(
                    wx_t[:, r * S : (r + 1) * S],
                    w_t[:, r * S : (r + 1) * S],
                    x_t[:, r : r + 1],
                    None,
                    op0=OP.mult,
                )

            first = c == 0
            last = c == NCHUNK - 1
            for r in range(RPC):
                # float32r: same f32 bits, PE full-rate mode (1 cyc/row at
                # N>=256, even-N required) vs 4 cyc/row for strict fp32.
                lhs = wx_t[:, r * S : (r + 1) * S]
                nc.tensor.matmul(acc[r][:], lhs, vt[r][:], start=first, stop=last)

        # --- finalize: y = num / max(den, 1e-30) -----------------------
        fpool = ctx.enter_context(tc.tile_pool(name="fin", bufs=2))
        ypool = ctx.enter_context(tc.tile_pool(name="yout", bufs=2))
        for r in range(RPC):
            den_c = fpool.tile([S, 1], F32, tag="den")
            nc.vector.tensor_scalar(
                den_c[:], acc[r][:, D : D + 1], 1e-30, None, op0=OP.max
            )
            rec = fpool.tile([S, 1], F32, tag="rec")
            nc.vector.reciprocal(rec[:], den_c[:])
            y_sb = ypool.tile([S, D], F32, tag="y")
            nc.vector.tensor_scalar(
                y_sb[:], acc[r][:, 0:D], rec[:], None, op0=OP.mult
            )
            nc.sync.dma_start(y_d[r, :, :], y_sb[:])


_NC_CACHE = None


def _get_nc():
    global _NC_CACHE
    if _NC_CACHE is None:
        _NC_CACHE = build_nc()
    return _NC_CACHE


def make_in_maps(Q, K, V, seps):
    Q = np.ascontiguousarray(np.asarray(Q, dtype=np.float32)).reshape(B, D)
    K = np.ascontiguousarray(np.asarray(K, dtype=np.float32))
    V = np.ascontiguousarray(np.asarray(V, dtype=np.float32))
    seps = np.ascontiguousarray(np.asarray(seps, dtype=np.int64))
    in_maps = []
    for i in range(N_CORES):
        sl = slice(i * RPC, (i + 1) * RPC)
        in_maps.append(
            {
                "Q": np.ascontiguousarray(Q[sl]),
                "K": np.ascontiguousarray(K[sl]),
                "V": np.ascontiguousarray(V[sl]),
                "seps": np.ascontiguousarray(seps[sl]).view(np.int32).reshape(1, -1),
            }
        )
    return in_maps


def kernel(Q, K, V, seps):
    nc = _get_nc()
    in_maps = make_in_maps(Q, K, V, seps)
    res = run_bass_kernel_spmd(nc, in_maps, core_ids=list(range(N_CORES)))
    y = np.concatenate([res.results[i]["y"] for i in range(N_CORES)], axis=0)
    y_mask = np.ones((B, S), dtype=np.float32)
    return (y, y_mask)


# revision 10
# speedup vs baseline: 1.6315x; 1.6315x over previous
"""Segmented (ragged) single-query attention on 8 TRN2 NeuronCores.

Problem: B=32 batch rows, each with one query q[256], keys/values K/V[4096,256]
and 64 sorted separator positions. Segment i of row b covers key positions p
with seps[b,i] < p < seps[b,i+1]; softmax attention is computed independently
per segment. Output y[32,63,256] (+ y_mask of ones).

Sharding: data-parallel over B - each of the 8 cores handles 4 batch rows,
no cross-core communication.

Per-core algorithm (r = 0..3 rows, c = 0..31 chunks of 128 key positions):
  e[p]  = (K[p,:] . q) / 16          one fused DVE scalar_tensor_tensor per
                                     [128,256] K-tile with accum_out (row-sum)
  x     = exp(e)                     ScalarE; no max-subtraction needed:
                                     e ~ N(0,1), overflow impossible (|e|<~25)
  M^T[p,i] = (pos_p > seps[i])*(pos_p < seps[i+1])   A-compare on DVE,
                                     combine on GpSimd, [128, 4*63] tiles
  W = M^T * x                        per-row scale on ScalarE (-> float32r)
  num[63,256] += W_r.T @ [V|1]_tile  PE float32r matmul (1 cyc/row), natural
  den[63,1]   (ones col of V tile)   V layout, PSUM accumulation over chunks
  y = num * 1/max(den, 1e-30)
Softmax weights are invariant to the max-shift, so skipping it is exact up to
fp rounding; empty segments give num=0, den=0 -> y=0, matching the reference.

DMA: the two HWDGE rings split the streams (K on sync/SP, V on scalar/ACT),
two 128-row chunks batched per dma_start; V lands in 8 manually-rotated
persistent tiles whose den-ones columns are written once at setup.
"""

import sys

for _p in ("/opt/trn_rl_repo",):
    if _p not in sys.path:
        sys.path.insert(0, _p)

import numpy as np

import concourse.bass as bass
import concourse.tile as tile
from concourse import bacc, mybir
from concourse.bass_utils import run_bass_kernel_spmd

N_CORES = 8
B, LK, D, NSEP = 32, 4096, 256, 64
S = NSEP - 1  # 63 segments
RPC = B // N_CORES  # rows per core = 4
NCHUNK = LK // 128  # 32
CB = 2  # chunks batched per DMA
NV = D + 2  # V tile row: 256 V cols + ones col (den) + pad col (f32r even-N)
F32 = mybir.dt.float32
F32R = mybir.dt.float32r
I32 = mybir.dt.int32
OP = mybir.AluOpType
AF = mybir.ActivationFunctionType


def build_nc():
    nc = bacc.Bacc(
        "TRN2",
        target_bir_lowering=False,
        debug=False,
        enable_asserts=False,
        num_devices=N_CORES,
    )
    q_d = nc.dram_tensor("Q", [RPC, D], F32, kind="ExternalInput").ap()
    k_d = nc.dram_tensor("K", [RPC, LK, D], F32, kind="ExternalInput").ap()
    v_d = nc.dram_tensor("V", [RPC, LK, D], F32R, kind="ExternalInput").ap()
    # seps int64 bytes passed as int32 pairs (little-endian; values < 4096 so
    # the low word at even indices is the value).
    s_d = nc.dram_tensor("seps", [1, 2 * RPC * NSEP], I32, kind="ExternalInput").ap()
    y_d = nc.dram_tensor("y", [RPC, S, D], F32, kind="ExternalOutput").ap()

    with tile.TileContext(nc) as tc:
        build_tile_kernel(tc, q_d, k_d, v_d, s_d, y_d)

    nc.compile()
    return nc


def build_tile_kernel(tc, q_d, k_d, v_d, s_d, y_d):
    nc = tc.nc
    from contextlib import ExitStack

    with ExitStack() as ctx:
        const = ctx.enter_context(tc.tile_pool(name="const", bufs=1))

        # --- constants -------------------------------------------------
        ones_col = const.tile([128, 1], F32, tag="ones_col")
        nc.vector.memset(ones_col[:], 1.0)
        ones_row = const.tile([1, 128], F32, tag="ones_row")
        nc.vector.memset(ones_row[:], 1.0)

        pos_i = const.tile([128, NCHUNK], I32, tag="pos_i")
        nc.gpsimd.iota(pos_i[:], pattern=[[128, NCHUNK]], base=0, channel_multiplier=1)
        pos_f = const.tile([128, NCHUNK], F32, tag="pos_f")
        nc.vector.tensor_copy(pos_f[:], pos_i[:])

        # --- seps -> lo/hi rows, broadcast across partitions -----------
        seps_raw = const.tile([1, 2 * RPC * NSEP], I32, tag="seps_raw")
        nc.sync.dma_start(seps_raw[:], s_d[:])
        seps_f = const.tile([1, RPC * NSEP], F32, tag="seps_f")
        raw_pairs = seps_raw[:].rearrange("p (n two) -> p n two", two=2)
        nc.vector.tensor_copy(
            seps_f[:].rearrange("p (n one) -> p n one", one=1),
            raw_pairs[:, :, 0:1],
        )
        # packed [1, 504]: cols 0:252 = lo (seps[r, 0:63]), 252:504 = hi
        # (seps[r, 1:64]), r-major blocks of 63.
        packed = const.tile([1, 2 * RPC * S], F32, tag="packed")
        by_row = seps_f[:].rearrange("p (r i) -> p r i", i=NSEP)
        nc.vector.tensor_copy(
            packed[:, 0 : RPC * S].rearrange("p (r i) -> p r i", i=S),
            by_row[:, :, 0:S],
        )
        nc.vector.tensor_copy(
            packed[:, RPC * S : 2 * RPC * S].rearrange("p (r i) -> p r i", i=S),
            by_row[:, :, 1 : S + 1],
        )

        with tc.tile_pool(name="setup_ps", bufs=2, space="PSUM") as setup_ps:
            lohi_ps = setup_ps.tile([128, 2 * RPC * S], F32, tag="lohi_ps")
            nc.tensor.matmul(lohi_ps[:], ones_row[:], packed[:], start=True, stop=True)
            lohi = const.tile([128, 2 * RPC * S], F32, tag="lohi")
            nc.vector.tensor_copy(lohi[:], lohi_ps[:])
            lo_b = lohi[:, 0 : RPC * S]
            hi_b = lohi[:, RPC * S : 2 * RPC * S]

            # --- q broadcast to all 128 partitions, one tile per row ---
            qb = []
            for r in range(RPC):
                q_sb = const.tile([1, D], F32, tag=f"q_sb{r}", name=f"q_sb{r}")
                nc.sync.dma_start(q_sb[:], q_d[r : r + 1, :])
                q_ps = setup_ps.tile(
                    [128, D], F32, tag=f"q_ps{r % 2}", name=f"q_ps{r}"
                )
                nc.tensor.matmul(q_ps[:], ones_row[:], q_sb[:], start=True, stop=True)
                qb_r = const.tile([128, D], F32, tag=f"qb{r}", name=f"qb{r}")
                nc.vector.tensor_copy(qb_r[:], q_ps[:])
                qb.append(qb_r)

        # --- persistent V tiles (manual rotation), ones cols pre-set ---
        NVBUF = 8
        vtiles = []
        for i in range(NVBUF):
            vt = const.tile([128, CB * NV], F32R, tag=f"vt{i}", name=f"vt{i}")
            vtiles.append(vt)
        # write the den-ones (+pad) columns once per buffer
        for i in range(NVBUF):
            for j in range(CB):
                nc.vector.tensor_copy(
                    vtiles[i][:, j * NV + D : j * NV + D + 1], ones_col[:]
                )
                nc.vector.tensor_copy(
                    vtiles[i][:, j * NV + D + 1 : j * NV + D + 2], ones_col[:]
                )

        # --- per-row PSUM accumulators [63, 258] (num | den | pad) -----
        acc_pool = ctx.enter_context(tc.tile_pool(name="acc", bufs=1, space="PSUM"))
        acc = [
            acc_pool.tile([S, D + 2], F32, tag=f"acc{r}", name=f"acc{r}")
            for r in range(RPC)
        ]

        kpool = ctx.enter_context(tc.tile_pool(name="kp", bufs=8))
        spool = ctx.enter_context(tc.tile_pool(name="scratch", bufs=8))
        epool = ctx.enter_context(tc.tile_pool(name="ep", bufs=6))
        wpool = ctx.enter_context(tc.tile_pool(name="wp", bufs=6))

        # --- main loop: 16 DMA super-steps of CB=2 chunks --------------
        vi = 0
        for cc in range(NCHUNK // CB):
            kt = []
            vt = []
            for r in range(RPC):
                k_t = kpool.tile([128, CB * D], F32, tag=f"k{r}", name=f"k{cc}_{r}")
                src = k_d[r, CB * 128 * cc : CB * 128 * (cc + 1), :].rearrange(
                    "(c p) d -> p c d", p=128
                )
                nc.sync.dma_start(k_t[:].rearrange("p (c d) -> p c d", d=D), src)
                kt.append(k_t)

                v_t = vtiles[vi % NVBUF]
                vi += 1
                vsrc = v_d[r, CB * 128 * cc : CB * 128 * (cc + 1), :].rearrange(
                    "(c p) d -> p c d", p=128
                )
                nc.scalar.dma_start(
                    v_t[:].rearrange("p (c n) -> p c n", n=NV)[:, :, 0:D], vsrc
                )
                vt.append(v_t)

            for j in range(CB):
                c = CB * cc + j
                pos_c = pos_f[:, c : c + 1]

                e_t = epool.tile([128, RPC], F32, tag="e", name=f"e{c}")
                for r in range(RPC):
                    scr = spool.tile([128, D], F32, tag="scr", name=f"scr{c}_{r}")
                    nc.vector.scalar_tensor_tensor(
                        scr[:],
                        kt[r][:, j * D : (j + 1) * D],
                        1.0 / 16.0,
                        qb[r][:],
                        op0=OP.mult,
                        op1=OP.mult,
                        accum_out=e_t[:, r : r + 1],
                    )
                x_t = epool.tile([128, RPC], F32, tag="x", name=f"x{c}")
                nc.scalar.activation(x_t[:], e_t[:], AF.Exp)

                a_t = wpool.tile([128, RPC * S], F32, tag="a", name=f"a{c}")
                nc.vector.tensor_scalar(a_t[:], lo_b, pos_c, None, op0=OP.is_lt)
                b_t = wpool.tile([128, RPC * S], F32, tag="b", name=f"b{c}")
                nc.vector.tensor_scalar(b_t[:], hi_b, pos_c, None, op0=OP.is_gt)
                w_t = wpool.tile([128, RPC * S], F32, tag="w", name=f"w{c}")
                nc.gpsimd.tensor_tensor(w_t[:], a_t[:], b_t[:], op=OP.mult)
                wx_t = wpool.tile([128, RPC * S], F32R, tag="wx", name=f"wx{c}")
                for r in range(RPC):
                    nc.scalar.activation(
                        wx_t[:, r * S : (r + 1) * S],
                        w_t[:, r * S : (r + 1) * S],
                        AF.Copy,
                        scale=x_t[:, r : r + 1],
                    )

                first = c == 0
                last = c == NCHUNK - 1
                for r in range(RPC):
                    lhs = wx_t[:, r * S : (r + 1) * S]
                    rhs = vt[r][:, j * NV : (j + 1) * NV]
                    nc.tensor.matmul(acc[r][:], lhs, rhs, start=first, stop=last)

        # --- finalize: y = num / max(den, 1e-30) -----------------------
        fpool = ctx.enter_context(tc.tile_pool(name="fin", bufs=2))
        ypool = ctx.enter_context(tc.tile_pool(name="yout", bufs=2))
        for r in range(RPC):
            den_c = fpool.tile([S, 1], F32, tag="den", name=f"den{r}")
            nc.vector.tensor_scalar(
                den_c[:], acc[r][:, D : D + 1], 1e-30, None, op0=OP.max
            )
            rec = fpool.tile([S, 1], F32, tag="rec", name=f"rec{r}")
            nc.vector.reciprocal(rec[:], den_c[:])
            y_sb = ypool.tile([S, D], F32, tag="y", name=f"y{r}")
            nc.vector.tensor_scalar(y_sb[:], acc[r][:, 0:D], rec[:], None, op0=OP.mult)
            nc.sync.dma_start(y_d[r, :, :], y_sb[:])


_NC_CACHE = None


def _get_nc():
    global _NC_CACHE
    if _NC_CACHE is None:
        _NC_CACHE = build_nc()
    return _NC_CACHE


def make_in_maps(Q, K, V, seps):
    Q = np.ascontiguousarray(np.asarray(Q, dtype=np.float32)).reshape(B, D)
    K = np.ascontiguousarray(np.asarray(K, dtype=np.float32))
    V = np.ascontiguousarray(np.asarray(V, dtype=np.float32))
    seps = np.ascontiguousarray(np.asarray(seps, dtype=np.int64))
    in_maps = []
    for i in range(N_CORES):
        sl = slice(i * RPC, (i + 1) * RPC)
        in_maps.append(
            {
                "Q": np.ascontiguousarray(Q[sl]),
                "K": np.ascontiguousarray(K[sl]),
                "V": np.ascontiguousarray(V[sl]),
                "seps": np.ascontiguousarray(seps[sl]).view(np.int32).reshape(1, -1),
            }
        )
    return in_maps


def kernel(Q, K, V, seps):
    nc = _get_nc()
    in_maps = make_in_maps(Q, K, V, seps)
    res = run_bass_kernel_spmd(nc, in_maps, core_ids=list(range(N_CORES)))
    y = np.concatenate([res.results[i]["y"] for i in range(N_CORES)], axis=0)
    y_mask = np.ones((B, S), dtype=np.float32)
    return (y, y_mask)


# revision 11
# speedup vs baseline: 1.6326x; 1.0007x over previous
"""Segmented (ragged) single-query attention on 8 TRN2 NeuronCores.

Problem: B=32 batch rows, each with one query q[256], keys/values K/V[4096,256]
and 64 sorted separator positions. Segment i of row b covers key positions p
with seps[b,i] < p < seps[b,i+1]; softmax attention is computed independently
per segment. Output y[32,63,256] (+ y_mask of ones).

Sharding: data-parallel over B - each of the 8 cores handles 4 batch rows,
no cross-core communication.

Per-core algorithm (r = 0..3 rows, c = 0..31 chunks of 128 key positions):
  e[p]  = (K[p,:] . q) / 16          one fused DVE scalar_tensor_tensor per
                                     [128,256] K-tile with accum_out (row-sum)
  x     = exp(e)                     ScalarE; no max-subtraction needed:
                                     e ~ N(0,1), overflow impossible (|e|<~25)
  M^T[p,i] = (pos_p > seps[i])*(pos_p < seps[i+1])   A-compare on DVE,
                                     combine on GpSimd, [128, 4*63] tiles
  W = M^T * x                        per-row scale on ScalarE (-> float32r)
  num[63,256] += W_r.T @ [V|1]_tile  PE float32r matmul (1 cyc/row), natural
  den[63,1]   (ones col of V tile)   V layout, PSUM accumulation over chunks
  y = num * 1/max(den, 1e-30)
Softmax weights are invariant to the max-shift, so skipping it is exact up to
fp rounding; empty segments give num=0, den=0 -> y=0, matching the reference.

DMA: the two HWDGE rings split the streams (K on sync/SP, V on scalar/ACT),
two 128-row chunks batched per dma_start; V lands in 8 manually-rotated
persistent tiles whose den-ones columns are written once at setup.
"""

import sys

for _p in ("/opt/trn_rl_repo",):
    if _p not in sys.path:
        sys.path.insert(0, _p)

import numpy as np

import concourse.bass as bass
import concourse.tile as tile
from concourse import bacc, mybir
from concourse.bass_utils import run_bass_kernel_spmd

N_CORES = 8
B, LK, D, NSEP = 32, 4096, 256, 64
S = NSEP - 1  # 63 segments
RPC = B // N_CORES  # rows per core = 4
NCHUNK = LK // 128  # 32
CB = 2  # chunks batched per DMA
NV = D + 2  # V tile row: 256 V cols + ones col (den) + pad col (f32r even-N)
F32 = mybir.dt.float32
F32R = mybir.dt.float32r
I32 = mybir.dt.int32
OP = mybir.AluOpType
AF = mybir.ActivationFunctionType


def build_nc():
    nc = bacc.Bacc(
        "TRN2",
        target_bir_lowering=False,
        debug=False,
        enable_asserts=False,
        num_devices=N_CORES,
    )
    q_d = nc.dram_tensor("Q", [RPC, D], F32, kind="ExternalInput").ap()
    k_d = nc.dram_tensor("K", [RPC, LK, D], F32, kind="ExternalInput").ap()
    v_d = nc.dram_tensor("V", [RPC, LK, D], F32R, kind="ExternalInput").ap()
    # seps int64 bytes passed as int32 pairs (little-endian; values < 4096 so
    # the low word at even indices is the value).
    s_d = nc.dram_tensor("seps", [1, 2 * RPC * NSEP], I32, kind="ExternalInput").ap()
    y_d = nc.dram_tensor("y", [RPC, S, D], F32, kind="ExternalOutput").ap()

    with tile.TileContext(nc) as tc:
        build_tile_kernel(tc, q_d, k_d, v_d, s_d, y_d)

    nc.compile()
    return nc


def build_tile_kernel(tc, q_d, k_d, v_d, s_d, y_d):
    nc = tc.nc
    from contextlib import ExitStack

    with ExitStack() as ctx:
        const = ctx.enter_context(tc.tile_pool(name="const", bufs=1))

        # --- constants -------------------------------------------------
        ones_col = const.tile([128, 1], F32, tag="ones_col")
        nc.vector.memset(ones_col[:], 1.0)
        ones_row = const.tile([1, 128], F32, tag="ones_row")
        nc.vector.memset(ones_row[:], 1.0)
        scale_row = const.tile([1, 128], F32, tag="scale_row")
        nc.vector.memset(scale_row[:], 1.0 / 16.0)

        pos_i = const.tile([128, NCHUNK], I32, tag="pos_i")
        nc.gpsimd.iota(pos_i[:], pattern=[[128, NCHUNK]], base=0, channel_multiplier=1)
        pos_f = const.tile([128, NCHUNK], F32, tag="pos_f")
        nc.vector.tensor_copy(pos_f[:], pos_i[:])

        # --- seps -> lo/hi rows, broadcast across partitions -----------
        seps_raw = const.tile([1, 2 * RPC * NSEP], I32, tag="seps_raw")
        nc.sync.dma_start(seps_raw[:], s_d[:])
        seps_f = const.tile([1, RPC * NSEP], F32, tag="seps_f")
        raw_pairs = seps_raw[:].rearrange("p (n two) -> p n two", two=2)
        nc.vector.tensor_copy(
            seps_f[:].rearrange("p (n one) -> p n one", one=1),
            raw_pairs[:, :, 0:1],
        )
        # packed [1, 504]: cols 0:252 = lo (seps[r, 0:63]), 252:504 = hi
        # (seps[r, 1:64]), r-major blocks of 63.
        packed = const.tile([1, 2 * RPC * S], F32, tag="packed")
        by_row = seps_f[:].rearrange("p (r i) -> p r i", i=NSEP)
        nc.vector.tensor_copy(
            packed[:, 0 : RPC * S].rearrange("p (r i) -> p r i", i=S),
            by_row[:, :, 0:S],
        )
        nc.vector.tensor_copy(
            packed[:, RPC * S : 2 * RPC * S].rearrange("p (r i) -> p r i", i=S),
            by_row[:, :, 1 : S + 1],
        )

        with tc.tile_pool(name="setup_ps", bufs=2, space="PSUM") as setup_ps:
            lohi_ps = setup_ps.tile([128, 2 * RPC * S], F32, tag="lohi_ps")
            nc.tensor.matmul(lohi_ps[:], ones_row[:], packed[:], start=True, stop=True)
            lohi = const.tile([128, 2 * RPC * S], F32, tag="lohi")
            nc.vector.tensor_copy(lohi[:], lohi_ps[:])
            lo_b = lohi[:, 0 : RPC * S]
            hi_b = lohi[:, RPC * S : 2 * RPC * S]

            # --- q broadcast to all 128 partitions, one tile per row ---
            qb = []
            for r in range(RPC):
                q_sb = const.tile([1, D], F32, tag=f"q_sb{r}", name=f"q_sb{r}")
                nc.sync.dma_start(q_sb[:], q_d[r : r + 1, :])
                q_ps = setup_ps.tile(
                    [128, D], F32, tag=f"q_ps{r % 2}", name=f"q_ps{r}"
                )
                nc.tensor.matmul(q_ps[:], scale_row[:], q_sb[:], start=True, stop=True)
                qb_r = const.tile([128, D], F32, tag=f"qb{r}", name=f"qb{r}")
                nc.vector.tensor_copy(qb_r[:], q_ps[:])
                qb.append(qb_r)

        # --- persistent V tiles (manual rotation), ones cols pre-set ---
        NVBUF = 8
        vtiles = []
        for i in range(NVBUF):
            vt = const.tile([128, CB * NV], F32R, tag=f"vt{i}", name=f"vt{i}")
            vtiles.append(vt)
        # write the den-ones (+pad) columns once per buffer
        for i in range(NVBUF):
            for j in range(CB):
                nc.vector.tensor_copy(
                    vtiles[i][:, j * NV + D : j * NV + D + 1], ones_col[:]
                )
                nc.vector.tensor_copy(
                    vtiles[i][:, j * NV + D + 1 : j * NV + D + 2], ones_col[:]
                )

        # --- per-row PSUM accumulators [63, 258] (num | den | pad) -----
        acc_pool = ctx.enter_context(tc.tile_pool(name="acc", bufs=1, space="PSUM"))
        acc = [
            acc_pool.tile([S, D + 2], F32, tag=f"acc{r}", name=f"acc{r}")
            for r in range(RPC)
        ]

        kpool = ctx.enter_context(tc.tile_pool(name="kp", bufs=8))
        spool = ctx.enter_context(tc.tile_pool(name="scratch", bufs=8))
        epool = ctx.enter_context(tc.tile_pool(name="ep", bufs=6))
        wpool = ctx.enter_context(tc.tile_pool(name="wp", bufs=6))

        # --- main loop: 16 DMA super-steps of CB=2 chunks --------------
        vi = 0
        for cc in range(NCHUNK // CB):
            kt = []
            vt = []
            for r in range(RPC):
                k_t = kpool.tile([128, CB * D], F32, tag=f"k{r}", name=f"k{cc}_{r}")
                src = k_d[r, CB * 128 * cc : CB * 128 * (cc + 1), :].rearrange(
                    "(c p) d -> p c d", p=128
                )
                nc.sync.dma_start(k_t[:].rearrange("p (c d) -> p c d", d=D), src)
                kt.append(k_t)

                v_t = vtiles[vi % NVBUF]
                vi += 1
                vsrc = v_d[r, CB * 128 * cc : CB * 128 * (cc + 1), :].rearrange(
                    "(c p) d -> p c d", p=128
                )
                v_eng = nc.scalar if r < 2 else nc.sync
                v_eng.dma_start(
                    v_t[:].rearrange("p (c n) -> p c n", n=NV)[:, :, 0:D], vsrc
                )
                vt.append(v_t)

            for j in range(CB):
                c = CB * cc + j
                pos_c = pos_f[:, c : c + 1]

                e_t = epool.tile([128, RPC], F32, tag="e", name=f"e{c}")
                for r in range(2):
                    scr = spool.tile([128, D], F32, tag="scr", name=f"scr{c}_{r}")
                    nc.vector.scalar_tensor_tensor(
                        scr[:],
                        kt[r][:, j * D : (j + 1) * D],
                        1.0,
                        qb[r][:],
                        op0=OP.mult,
                        op1=OP.mult,
                        accum_out=e_t[:, r : r + 1],
                    )
                for r in range(2, RPC):
                    prod = spool.tile([128, D], F32, tag="prod", name=f"prod{c}_{r}")
                    nc.gpsimd.tensor_tensor(
                        prod[:], kt[r][:, j * D : (j + 1) * D], qb[r][:], op=OP.mult
                    )
                    scr2 = spool.tile([128, D], F32, tag="scr2", name=f"scr2{c}_{r}")
                    nc.scalar.activation(
                        scr2[:], prod[:], AF.Copy, accum_out=e_t[:, r : r + 1]
                    )
                x_t = epool.tile([128, RPC], F32, tag="x", name=f"x{c}")
                nc.scalar.activation(x_t[:], e_t[:], AF.Exp)

                a_t = wpool.tile([128, RPC * S], F32, tag="a", name=f"a{c}")
                nc.vector.tensor_scalar(a_t[:], lo_b, pos_c, None, op0=OP.is_lt)
                w_t = wpool.tile([128, RPC * S], F32, tag="w", name=f"w{c}")
                nc.vector.scalar_tensor_tensor(
                    w_t[:], hi_b, pos_c, a_t[:], op0=OP.is_gt, op1=OP.mult
                )
                wx_t = wpool.tile([128, RPC * S], F32R, tag="wx", name=f"wx{c}")
                w_v = w_t[:].rearrange("p (r i) -> p r i", i=S)
                x_v = x_t[:].rearrange("p (r one) -> p r one", one=1)
                w_bc, x_bc = bass.broadcast_tensor_aps(w_v, x_v)
                nc.gpsimd.tensor_tensor(
                    wx_t[:].rearrange("p (r i) -> p r i", i=S), w_bc, x_bc, op=OP.mult
                )

                first = c == 0
                last = c == NCHUNK - 1
                for r in range(RPC):
                    lhs = wx_t[:, r * S : (r + 1) * S]
                    rhs = vt[r][:, j * NV : (j + 1) * NV]
                    nc.tensor.matmul(acc[r][:], lhs, rhs, start=first, stop=last)

        # --- finalize: y = num / max(den, 1e-30) -----------------------
        fpool = ctx.enter_context(tc.tile_pool(name="fin", bufs=2))
        ypool = ctx.enter_context(tc.tile_pool(name="yout", bufs=2))
        for r in range(RPC):
            den_c = fpool.tile([S, 1], F32, tag="den", name=f"den{r}")
            nc.vector.tensor_scalar(
                den_c[:], acc[r][:, D : D + 1], 1e-30, None, op0=OP.max
            )
            rec = fpool.tile([S, 1], F32, tag="rec", name=f"rec{r}")
            nc.vector.reciprocal(rec[:], den_c[:])
            y_sb = ypool.tile([S, D], F32, tag="y", name=f"y{r}")
            nc.vector.tensor_scalar(y_sb[:], acc[r][:, 0:D], rec[:], None, op0=OP.mult)
            nc.sync.dma_start(y_d[r, :, :], y_sb[:])


_NC_CACHE = None


def _get_nc():
    global _NC_CACHE
    if _NC_CACHE is None:
        _NC_CACHE = build_nc()
    return _NC_CACHE


def make_in_maps(Q, K, V, seps):
    Q = np.ascontiguousarray(np.asarray(Q, dtype=np.float32)).reshape(B, D)
    K = np.ascontiguousarray(np.asarray(K, dtype=np.float32))
    V = np.ascontiguousarray(np.asarray(V, dtype=np.float32))
    seps = np.ascontiguousarray(np.asarray(seps, dtype=np.int64))
    in_maps = []
    for i in range(N_CORES):
        sl = slice(i * RPC, (i + 1) * RPC)
        in_maps.append(
            {
                "Q": np.ascontiguousarray(Q[sl]),
                "K": np.ascontiguousarray(K[sl]),
                "V": np.ascontiguousarray(V[sl]),
                "seps": np.ascontiguousarray(seps[sl]).view(np.int32).reshape(1, -1),
            }
        )
    return in_maps


def kernel(Q, K, V, seps):
    nc = _get_nc()
    in_maps = make_in_maps(Q, K, V, seps)
    res = run_bass_kernel_spmd(nc, in_maps, core_ids=list(range(N_CORES)))
    y = np.concatenate([res.results[i]["y"] for i in range(N_CORES)], axis=0)
    y_mask = np.ones((B, S), dtype=np.float32)
    return (y, y_mask)


# revision 12
# speedup vs baseline: 1.6534x; 1.0127x over previous
"""Segmented (ragged) single-query attention on 8 TRN2 NeuronCores.

Problem: B=32 batch rows, each with one query q[256], keys/values K/V[4096,256]
and 64 sorted separator positions. Segment i of row b covers key positions p
with seps[b,i] < p < seps[b,i+1]; softmax attention is computed independently
per segment. Output y[32,63,256] (+ y_mask of ones).

Sharding: data-parallel over B - each of the 8 cores handles 4 batch rows,
no cross-core communication.

Per-core algorithm (r = 0..3 rows, c = 0..31 chunks of 128 key positions):
  e[p]  = (K[p,:] . q) / 16          one fused DVE scalar_tensor_tensor per
                                     [128,256] K-tile with accum_out (row-sum)
  x     = exp(e)                     ScalarE; no max-subtraction needed:
                                     e ~ N(0,1), overflow impossible (|e|<~25)
  M^T[p,i] = (pos_p > seps[i])*(pos_p < seps[i+1])   A-compare on DVE,
                                     combine on GpSimd, [128, 4*63] tiles
  W = M^T * x                        per-row scale on ScalarE (-> float32r)
  num[63,256] += W_r.T @ [V|1]_tile  PE float32r matmul (1 cyc/row), natural
  den[63,1]   (ones col of V tile)   V layout, PSUM accumulation over chunks
  y = num * 1/max(den, 1e-30)
Softmax weights are invariant to the max-shift, so skipping it is exact up to
fp rounding; empty segments give num=0, den=0 -> y=0, matching the reference.

DMA: the two HWDGE rings split the streams (K on sync/SP, V on scalar/ACT),
two 128-row chunks batched per dma_start; V lands in 8 manually-rotated
persistent tiles whose den-ones columns are written once at setup.
"""

import sys

for _p in ("/opt/trn_rl_repo",):
    if _p not in sys.path:
        sys.path.insert(0, _p)

import numpy as np

import concourse.bass as bass
import concourse.tile as tile
from concourse import bacc, mybir
from concourse.bass_utils import run_bass_kernel_spmd

N_CORES = 8
B, LK, D, NSEP = 32, 4096, 256, 64
S = NSEP - 1  # 63 segments
RPC = B // N_CORES  # rows per core = 4
NCHUNK = LK // 128  # 32
CB = 4  # chunks batched per DMA
NV = D + 2  # V tile row: 256 V cols + ones col (den) + pad col (f32r even-N)
F32 = mybir.dt.float32
F32R = mybir.dt.float32r
I32 = mybir.dt.int32
OP = mybir.AluOpType
AF = mybir.ActivationFunctionType


def build_nc():
    nc = bacc.Bacc(
        "TRN2",
        target_bir_lowering=False,
        debug=False,
        enable_asserts=False,
        num_devices=N_CORES,
    )
    q_d = nc.dram_tensor("Q", [RPC, D], F32, kind="ExternalInput").ap()
    k_d = nc.dram_tensor("K", [RPC, LK, D], F32, kind="ExternalInput").ap()
    v_d = nc.dram_tensor("V", [RPC, LK, D], F32R, kind="ExternalInput").ap()
    # seps int64 bytes passed as int32 pairs (little-endian; values < 4096 so
    # the low word at even indices is the value).
    s_d = nc.dram_tensor("seps", [1, 2 * RPC * NSEP], I32, kind="ExternalInput").ap()
    y_d = nc.dram_tensor("y", [RPC, S, D], F32, kind="ExternalOutput").ap()

    with tile.TileContext(nc) as tc:
        build_tile_kernel(tc, q_d, k_d, v_d, s_d, y_d)

    nc.compile()
    return nc


def build_tile_kernel(tc, q_d, k_d, v_d, s_d, y_d):
    nc = tc.nc
    from contextlib import ExitStack

    with ExitStack() as ctx:
        const = ctx.enter_context(tc.tile_pool(name="const", bufs=1))

        # --- constants -------------------------------------------------
        ones_col = const.tile([128, 1], F32, tag="ones_col")
        nc.vector.memset(ones_col[:], 1.0)
        ones_row = const.tile([1, 128], F32, tag="ones_row")
        nc.vector.memset(ones_row[:], 1.0)
        scale_row = const.tile([1, 128], F32, tag="scale_row")
        nc.vector.memset(scale_row[:], 1.0 / 16.0)

        pos_i = const.tile([128, NCHUNK], I32, tag="pos_i")
        nc.gpsimd.iota(pos_i[:], pattern=[[128, NCHUNK]], base=0, channel_multiplier=1)
        pos_f = const.tile([128, NCHUNK], F32, tag="pos_f")
        nc.vector.tensor_copy(pos_f[:], pos_i[:])

        # --- seps -> lo/hi rows, broadcast across partitions -----------
        seps_raw = const.tile([1, 2 * RPC * NSEP], I32, tag="seps_raw")
        nc.sync.dma_start(seps_raw[:], s_d[:])
        seps_f = const.tile([1, RPC * NSEP], F32, tag="seps_f")
        raw_pairs = seps_raw[:].rearrange("p (n two) -> p n two", two=2)
        nc.vector.tensor_copy(
            seps_f[:].rearrange("p (n one) -> p n one", one=1),
            raw_pairs[:, :, 0:1],
        )
        # packed [1, 504]: cols 0:252 = lo (seps[r, 0:63]), 252:504 = hi
        # (seps[r, 1:64]), r-major blocks of 63.
        packed = const.tile([1, 2 * RPC * S], F32, tag="packed")
        by_row = seps_f[:].rearrange("p (r i) -> p r i", i=NSEP)
        nc.vector.tensor_copy(
            packed[:, 0 : RPC * S].rearrange("p (r i) -> p r i", i=S),
            by_row[:, :, 0:S],
        )
        nc.vector.tensor_copy(
            packed[:, RPC * S : 2 * RPC * S].rearrange("p (r i) -> p r i", i=S),
            by_row[:, :, 1 : S + 1],
        )

        with tc.tile_pool(name="setup_ps", bufs=2, space="PSUM") as setup_ps:
            lohi_ps = setup_ps.tile([128, 2 * RPC * S], F32, tag="lohi_ps")
            nc.tensor.matmul(lohi_ps[:], ones_row[:], packed[:], start=True, stop=True)
            lohi = const.tile([128, 2 * RPC * S], F32, tag="lohi")
            nc.vector.tensor_copy(lohi[:], lohi_ps[:])
            lo_b = lohi[:, 0 : RPC * S]
            hi_b = lohi[:, RPC * S : 2 * RPC * S]

            # --- q broadcast to all 128 partitions, one tile per row ---
            qb = []
            for r in range(RPC):
                q_sb = const.tile([1, D], F32, tag=f"q_sb{r}", name=f"q_sb{r}")
                nc.sync.dma_start(q_sb[:], q_d[r : r + 1, :])
                q_ps = setup_ps.tile(
                    [128, D], F32, tag=f"q_ps{r % 2}", name=f"q_ps{r}"
                )
                nc.tensor.matmul(q_ps[:], scale_row[:], q_sb[:], start=True, stop=True)
                qb_r = const.tile([128, D], F32, tag=f"qb{r}", name=f"qb{r}")
                nc.vector.tensor_copy(qb_r[:], q_ps[:])
                qb.append(qb_r)

        # --- persistent V tiles (manual rotation), ones cols pre-set ---
        NVBUF = 8
        vtiles = []
        for i in range(NVBUF):
            vt = const.tile([128, CB * NV], F32R, tag=f"vt{i}", name=f"vt{i}")
            vtiles.append(vt)
        # write the den-ones (+pad) columns once per buffer
        for i in range(NVBUF):
            for j in range(CB):
                nc.vector.tensor_copy(
                    vtiles[i][:, j * NV + D : j * NV + D + 1], ones_col[:]
                )
                nc.vector.tensor_copy(
                    vtiles[i][:, j * NV + D + 1 : j * NV + D + 2], ones_col[:]
                )

        # --- per-row PSUM accumulators [63, 258] (num | den | pad) -----
        acc_pool = ctx.enter_context(tc.tile_pool(name="acc", bufs=1, space="PSUM"))
        acc = [
            acc_pool.tile([S, D + 2], F32, tag=f"acc{r}", name=f"acc{r}")
            for r in range(RPC)
        ]

        kpool = ctx.enter_context(tc.tile_pool(name="kp", bufs=6))
        spool = ctx.enter_context(tc.tile_pool(name="scratch", bufs=10))
        epool = ctx.enter_context(tc.tile_pool(name="ep", bufs=10))
        wpool = ctx.enter_context(tc.tile_pool(name="wp", bufs=10))

        # --- main loop: 16 DMA super-steps of CB=2 chunks --------------
        vi = 0
        for cc in range(NCHUNK // CB):
            kt = []
            vt = []
            for r in range(RPC):
                k_t = kpool.tile([128, CB * D], F32, tag=f"k{r}", name=f"k{cc}_{r}")
                src = k_d[r, CB * 128 * cc : CB * 128 * (cc + 1), :].rearrange(
                    "(c p) d -> p c d", p=128
                )
                nc.sync.dma_start(k_t[:].rearrange("p (c d) -> p c d", d=D), src)
                kt.append(k_t)

                v_t = vtiles[vi % NVBUF]
                vi += 1
                vsrc = v_d[r, CB * 128 * cc : CB * 128 * (cc + 1), :].rearrange(
                    "(c p) d -> p c d", p=128
                )
                v_eng = nc.scalar if r < 2 else nc.sync
                v_eng.dma_start(
                    v_t[:].rearrange("p (c n) -> p c n", n=NV)[:, :, 0:D], vsrc
                )
                vt.append(v_t)

            for j in range(CB):
                c = CB * cc + j
                pos_c = pos_f[:, c : c + 1]

                e_t = epool.tile([128, RPC], F32, tag="e", name=f"e{c}")
                for r in range(2):
                    scr = spool.tile([128, D], F32, tag="scr", name=f"scr{c}_{r}")
                    nc.vector.scalar_tensor_tensor(
                        scr[:],
                        kt[r][:, j * D : (j + 1) * D],
                        1.0,
                        qb[r][:],
                        op0=OP.mult,
                        op1=OP.mult,
                        accum_out=e_t[:, r : r + 1],
                    )
                for r in range(2, RPC):
                    prod = spool.tile([128, D], F32, tag="prod", name=f"prod{c}_{r}")
                    nc.gpsimd.tensor_tensor(
                        prod[:], kt[r][:, j * D : (j + 1) * D], qb[r][:], op=OP.mult
                    )
                    scr2 = spool.tile([128, D], F32, tag="scr2", name=f"scr2{c}_{r}")
                    nc.scalar.activation(
                        scr2[:], prod[:], AF.Copy, accum_out=e_t[:, r : r + 1]
                    )
                x_t = epool.tile([128, RPC], F32, tag="x", name=f"x{c}")
                nc.scalar.activation(x_t[:], e_t[:], AF.Exp)

                a_t = wpool.tile([128, RPC * S], F32, tag="a", name=f"a{c}")
                nc.vector.tensor_scalar(a_t[:], lo_b, pos_c, None, op0=OP.is_lt)
                w_t = wpool.tile([128, RPC * S], F32, tag="w", name=f"w{c}")
                nc.vector.scalar_tensor_tensor(
                    w_t[:], hi_b, pos_c, a_t[:], op0=OP.is_gt, op1=OP.mult
                )
                wx_t = wpool.tile([128, RPC * S], F32R, tag="wx", name=f"wx{c}")
                w_v = w_t[:].rearrange("p (r i) -> p r i", i=S)
                x_v = x_t[:].rearrange("p (r one) -> p r one", one=1)
                w_bc, x_bc = bass.broadcast_tensor_aps(w_v, x_v)
                nc.gpsimd.tensor_tensor(
                    wx_t[:].rearrange("p (r i) -> p r i", i=S), w_bc, x_bc, op=OP.mult
                )

                first = c == 0
                last = c == NCHUNK - 1
                for r in range(RPC):
                    lhs = wx_t[:, r * S : (r + 1) * S]
                    rhs = vt[r][:, j * NV : (j + 1) * NV]
                    nc.tensor.matmul(acc[r][:], lhs, rhs, start=first, stop=last)

        # --- finalize: y = num / max(den, 1e-30) -----------------------
        fpool = ctx.enter_context(tc.tile_pool(name="fin", bufs=2))
        ypool = ctx.enter_context(tc.tile_pool(name="yout", bufs=2))
        for r in range(RPC):
            den_c = fpool.tile([S, 1], F32, tag="den", name=f"den{r}")
            nc.vector.tensor_scalar(
                den_c[:], acc[r][:, D : D + 1], 1e-30, None, op0=OP.max
            )
            rec = fpool.tile([S, 1], F32, tag="rec", name=f"rec{r}")
            nc.vector.reciprocal(rec[:], den_c[:])
            y_sb = ypool.tile([S, D], F32, tag="y", name=f"y{r}")
            nc.vector.tensor_scalar(y_sb[:], acc[r][:, 0:D], rec[:], None, op0=OP.mult)
            nc.sync.dma_start(y_d[r, :, :], y_sb[:])


_NC_CACHE = None


def _get_nc():
    global _NC_CACHE
    if _NC_CACHE is None:
        _NC_CACHE = build_nc()
    return _NC_CACHE


def make_in_maps(Q, K, V, seps):
    Q = np.ascontiguousarray(np.asarray(Q, dtype=np.float32)).reshape(B, D)
    K = np.ascontiguousarray(np.asarray(K, dtype=np.float32))
    V = np.ascontiguousarray(np.asarray(V, dtype=np.float32))
    seps = np.ascontiguousarray(np.asarray(seps, dtype=np.int64))
    in_maps = []
    for i in range(N_CORES):
        sl = slice(i * RPC, (i + 1) * RPC)
        in_maps.append(
            {
                "Q": np.ascontiguousarray(Q[sl]),
                "K": np.ascontiguousarray(K[sl]),
                "V": np.ascontiguousarray(V[sl]),
                "seps": np.ascontiguousarray(seps[sl]).view(np.int32).reshape(1, -1),
            }
        )
    return in_maps


def kernel(Q, K, V, seps):
    nc = _get_nc()
    in_maps = make_in_maps(Q, K, V, seps)
    res = run_bass_kernel_spmd(nc, in_maps, core_ids=list(range(N_CORES)))
    y = np.concatenate([res.results[i]["y"] for i in range(N_CORES)], axis=0)
    y_mask = np.ones((B, S), dtype=np.float32)
    return (y, y_mask)


# revision 13
# speedup vs baseline: 1.6627x; 1.0056x over previous
"""Segmented (ragged) single-query attention on 8 TRN2 NeuronCores.

Problem: B=32 batch rows, each with one query q[256], keys/values K/V[4096,256]
and 64 sorted separator positions. Segment i of row b covers key positions p
with seps[b,i] < p < seps[b,i+1]; softmax attention is computed independently
per segment. Output y[32,63,256] (+ y_mask of ones).

Sharding: data-parallel over B - each of the 8 cores handles 4 batch rows,
no cross-core communication.

Per-core algorithm (r = 0..3 rows, c = 0..31 chunks of 128 key positions):
  e[p]  = (K[p,:] . q) / 16          one fused DVE scalar_tensor_tensor per
                                     [128,256] K-tile with accum_out (row-sum)
  x     = exp(e)                     ScalarE; no max-subtraction needed:
                                     e ~ N(0,1), overflow impossible (|e|<~25)
  M^T[p,i] = (pos_p > seps[i])*(pos_p < seps[i+1])   A-compare on DVE,
                                     combine on GpSimd, [128, 4*63] tiles
  W = M^T * x                        per-row scale on ScalarE (-> float32r)
  num[63,256] += W_r.T @ [V|1]_tile  PE float32r matmul (1 cyc/row), natural
  den[63,1]   (ones col of V tile)   V layout, PSUM accumulation over chunks
  y = num * 1/max(den, 1e-30)
Softmax weights are invariant to the max-shift, so skipping it is exact up to
fp rounding; empty segments give num=0, den=0 -> y=0, matching the reference.

DMA: the two HWDGE rings split the streams (K on sync/SP, V on scalar/ACT),
two 128-row chunks batched per dma_start; V lands in 8 manually-rotated
persistent tiles whose den-ones columns are written once at setup.
"""

import sys

for _p in ("/opt/trn_rl_repo",):
    if _p not in sys.path:
        sys.path.insert(0, _p)

import numpy as np

import concourse.bass as bass
import concourse.tile as tile
from concourse import bacc, mybir
from concourse.bass_utils import run_bass_kernel_spmd

N_CORES = 8
B, LK, D, NSEP = 32, 4096, 256, 64
S = NSEP - 1  # 63 segments
RPC = B // N_CORES  # rows per core = 4
NCHUNK = LK // 128  # 32
CB = 4  # chunks batched per DMA
NV = D + 2  # V tile row: 256 V cols + ones col (den) + pad col (f32r even-N)
F32 = mybir.dt.float32
F32R = mybir.dt.float32r
BF16 = mybir.dt.bfloat16
I32 = mybir.dt.int32
OP = mybir.AluOpType
AF = mybir.ActivationFunctionType


def build_nc():
    nc = bacc.Bacc(
        "TRN2",
        target_bir_lowering=False,
        debug=False,
        enable_asserts=False,
        num_devices=N_CORES,
    )
    q_d = nc.dram_tensor("Q", [RPC, D], F32, kind="ExternalInput").ap()
    k_d = nc.dram_tensor("K", [RPC, LK, D], F32, kind="ExternalInput").ap()
    v_d = nc.dram_tensor("V", [RPC, LK, D], F32R, kind="ExternalInput").ap()
    # seps int64 bytes passed as int32 pairs (little-endian; values < 4096 so
    # the low word at even indices is the value).
    s_d = nc.dram_tensor("seps", [1, 2 * RPC * NSEP], I32, kind="ExternalInput").ap()
    y_d = nc.dram_tensor("y", [RPC, S, D], F32, kind="ExternalOutput").ap()

    with tile.TileContext(nc) as tc:
        build_tile_kernel(tc, q_d, k_d, v_d, s_d, y_d)

    nc.compile()
    return nc


def build_tile_kernel(tc, q_d, k_d, v_d, s_d, y_d):
    nc = tc.nc
    from contextlib import ExitStack

    with ExitStack() as ctx:
        const = ctx.enter_context(tc.tile_pool(name="const", bufs=1))

        # --- constants -------------------------------------------------
        ones_col = const.tile([128, 1], F32, tag="ones_col")
        nc.vector.memset(ones_col[:], 1.0)
        ones_row = const.tile([1, 128], F32, tag="ones_row")
        nc.vector.memset(ones_row[:], 1.0)
        scale_row = const.tile([1, 128], F32, tag="scale_row")
        nc.vector.memset(scale_row[:], 1.0 / 16.0)

        pos_i = const.tile([128, NCHUNK], I32, tag="pos_i")
        nc.gpsimd.iota(pos_i[:], pattern=[[128, NCHUNK]], base=0, channel_multiplier=1)
        pos_f = const.tile([128, NCHUNK], F32, tag="pos_f")
        nc.vector.tensor_copy(pos_f[:], pos_i[:])

        # --- seps -> lo/hi rows, broadcast across partitions -----------
        seps_raw = const.tile([1, 2 * RPC * NSEP], I32, tag="seps_raw")
        nc.sync.dma_start(seps_raw[:], s_d[:])
        seps_f = const.tile([1, RPC * NSEP], F32, tag="seps_f")
        raw_pairs = seps_raw[:].rearrange("p (n two) -> p n two", two=2)
        nc.vector.tensor_copy(
            seps_f[:].rearrange("p (n one) -> p n one", one=1),
            raw_pairs[:, :, 0:1],
        )
        # packed [1, 504]: cols 0:252 = lo (seps[r, 0:63]), 252:504 = hi
        # (seps[r, 1:64]), r-major blocks of 63.
        packed = const.tile([1, 2 * RPC * S], F32, tag="packed")
        by_row = seps_f[:].rearrange("p (r i) -> p r i", i=NSEP)
        nc.vector.tensor_copy(
            packed[:, 0 : RPC * S].rearrange("p (r i) -> p r i", i=S),
            by_row[:, :, 0:S],
        )
        nc.vector.tensor_copy(
            packed[:, RPC * S : 2 * RPC * S].rearrange("p (r i) -> p r i", i=S),
            by_row[:, :, 1 : S + 1],
        )

        with tc.tile_pool(name="setup_ps", bufs=2, space="PSUM") as setup_ps:
            lohi_ps = setup_ps.tile([128, 2 * RPC * S], F32, tag="lohi_ps")
            nc.tensor.matmul(lohi_ps[:], ones_row[:], packed[:], start=True, stop=True)
            lohi = const.tile([128, 2 * RPC * S], F32, tag="lohi")
            nc.vector.tensor_copy(lohi[:], lohi_ps[:])
            lo_b = lohi[:, 0 : RPC * S]
            hi_b = lohi[:, RPC * S : 2 * RPC * S]

            # --- q broadcast to all 128 partitions, one tile per row ---
            qb = []
            for r in range(RPC):
                q_sb = const.tile([1, D], F32, tag=f"q_sb{r}", name=f"q_sb{r}")
                nc.sync.dma_start(q_sb[:], q_d[r : r + 1, :])
                q_ps = setup_ps.tile(
                    [128, D], F32, tag=f"q_ps{r % 2}", name=f"q_ps{r}"
                )
                nc.tensor.matmul(q_ps[:], scale_row[:], q_sb[:], start=True, stop=True)
                qb_r = const.tile([128, D], F32, tag=f"qb{r}", name=f"qb{r}")
                nc.vector.tensor_copy(qb_r[:], q_ps[:])
                qb.append(qb_r)

        # --- persistent V tiles (manual rotation), ones cols pre-set ---
        NVBUF = 8
        vtiles = []
        for i in range(NVBUF):
            vt = const.tile([128, CB * NV], F32R, tag=f"vt{i}", name=f"vt{i}")
            vtiles.append(vt)
        # write the den-ones (+pad) columns once per buffer
        for i in range(NVBUF):
            for j in range(CB):
                nc.vector.tensor_copy(
                    vtiles[i][:, j * NV + D : j * NV + D + 1], ones_col[:]
                )
                nc.vector.tensor_copy(
                    vtiles[i][:, j * NV + D + 1 : j * NV + D + 2], ones_col[:]
                )

        # --- per-row PSUM accumulators [63, 258] (num | den | pad) -----
        acc_pool = ctx.enter_context(tc.tile_pool(name="acc", bufs=1, space="PSUM"))
        acc = [
            acc_pool.tile([S, D + 2], F32, tag=f"acc{r}", name=f"acc{r}")
            for r in range(RPC)
        ]

        kpool = ctx.enter_context(tc.tile_pool(name="kp", bufs=6))
        spool = ctx.enter_context(tc.tile_pool(name="scratch", bufs=10))
        epool = ctx.enter_context(tc.tile_pool(name="ep", bufs=10))
        wpool = ctx.enter_context(tc.tile_pool(name="wp", bufs=10))

        # --- main loop: 16 DMA super-steps of CB=2 chunks --------------
        vi = 0
        for cc in range(NCHUNK // CB):
            kt = []
            vt = []
            for r in range(RPC):
                k_t = kpool.tile([128, CB * D], F32, tag=f"k{r}", name=f"k{cc}_{r}")
                src = k_d[r, CB * 128 * cc : CB * 128 * (cc + 1), :].rearrange(
                    "(c p) d -> p c d", p=128
                )
                nc.sync.dma_start(k_t[:].rearrange("p (c d) -> p c d", d=D), src)
                kt.append(k_t)

                v_t = vtiles[vi % NVBUF]
                vi += 1
                vsrc = v_d[r, CB * 128 * cc : CB * 128 * (cc + 1), :].rearrange(
                    "(c p) d -> p c d", p=128
                )
                v_eng = nc.scalar if r < 2 else nc.sync
                v_eng.dma_start(
                    v_t[:].rearrange("p (c n) -> p c n", n=NV)[:, :, 0:D], vsrc
                )
                vt.append(v_t)

            for j in range(CB):
                c = CB * cc + j
                pos_c = pos_f[:, c : c + 1]

                e_t = epool.tile([128, RPC], F32, tag="e", name=f"e{c}")
                for r in range(2):
                    scr = spool.tile([128, D], BF16, tag="scr", name=f"scr{c}_{r}")
                    nc.vector.scalar_tensor_tensor(
                        scr[:],
                        kt[r][:, j * D : (j + 1) * D],
                        1.0,
                        qb[r][:],
                        op0=OP.mult,
                        op1=OP.mult,
                        accum_out=e_t[:, r : r + 1],
                    )
                for r in range(2, RPC):
                    prod = spool.tile([128, D], F32, tag="prod", name=f"prod{c}_{r}")
                    nc.gpsimd.tensor_tensor(
                        prod[:], kt[r][:, j * D : (j + 1) * D], qb[r][:], op=OP.mult
                    )
                    scr2 = spool.tile([128, D], BF16, tag="scr2", name=f"scr2{c}_{r}")
                    nc.scalar.activation(
                        scr2[:], prod[:], AF.Copy, accum_out=e_t[:, r : r + 1]
                    )
                x_t = epool.tile([128, RPC], F32, tag="x", name=f"x{c}")
                nc.scalar.activation(x_t[:], e_t[:], AF.Exp)

                a_t = wpool.tile([128, RPC * S], F32, tag="a", name=f"a{c}")
                nc.vector.tensor_scalar(a_t[:], lo_b, pos_c, None, op0=OP.is_lt)
                w_t = wpool.tile([128, RPC * S], F32, tag="w", name=f"w{c}")
                nc.vector.scalar_tensor_tensor(
                    w_t[:], hi_b, pos_c, a_t[:], op0=OP.is_gt, op1=OP.mult
                )
                wx_t = wpool.tile([128, RPC * S], F32R, tag="wx", name=f"wx{c}")
                w_v = w_t[:].rearrange("p (r i) -> p r i", i=S)
                x_v = x_t[:].rearrange("p (r one) -> p r one", one=1)
                w_bc, x_bc = bass.broadcast_tensor_aps(w_v, x_v)
                nc.gpsimd.tensor_tensor(
                    wx_t[:].rearrange("p (r i) -> p r i", i=S), w_bc, x_bc, op=OP.mult
                )

                first = c == 0
                last = c == NCHUNK - 1
                for r in range(RPC):
                    lhs = wx_t[:, r * S : (r + 1) * S]
                    rhs = vt[r][:, j * NV : (j + 1) * NV]
                    nc.tensor.matmul(acc[r][:], lhs, rhs, start=first, stop=last)

        # --- finalize: y = num / max(den, 1e-30) -----------------------
        fpool = ctx.enter_context(tc.tile_pool(name="fin", bufs=2))
        ypool = ctx.enter_context(tc.tile_pool(name="yout", bufs=2))
        for r in range(RPC):
            den_c = fpool.tile([S, 1], F32, tag="den", name=f"den{r}")
            nc.vector.tensor_scalar(
                den_c[:], acc[r][:, D : D + 1], 1e-30, None, op0=OP.max
            )
            rec = fpool.tile([S, 1], F32, tag="rec", name=f"rec{r}")
            nc.vector.reciprocal(rec[:], den_c[:])
            y_sb = ypool.tile([S, D], F32, tag="y", name=f"y{r}")
            nc.vector.tensor_scalar(y_sb[:], acc[r][:, 0:D], rec[:], None, op0=OP.mult)
            nc.sync.dma_start(y_d[r, :, :], y_sb[:])


_NC_CACHE = None


def _get_nc():
    global _NC_CACHE
    if _NC_CACHE is None:
        _NC_CACHE = build_nc()
    return _NC_CACHE


def make_in_maps(Q, K, V, seps):
    Q = np.ascontiguousarray(np.asarray(Q, dtype=np.float32)).reshape(B, D)
    K = np.ascontiguousarray(np.asarray(K, dtype=np.float32))
    V = np.ascontiguousarray(np.asarray(V, dtype=np.float32))
    seps = np.ascontiguousarray(np.asarray(seps, dtype=np.int64))
    in_maps = []
    for i in range(N_CORES):
        sl = slice(i * RPC, (i + 1) * RPC)
        in_maps.append(
            {
                "Q": np.ascontiguousarray(Q[sl]),
                "K": np.ascontiguousarray(K[sl]),
                "V": np.ascontiguousarray(V[sl]),
                "seps": np.ascontiguousarray(seps[sl]).view(np.int32).reshape(1, -1),
            }
        )
    return in_maps


def kernel(Q, K, V, seps):
    nc = _get_nc()
    in_maps = make_in_maps(Q, K, V, seps)
    res = run_bass_kernel_spmd(nc, in_maps, core_ids=list(range(N_CORES)))
    y = np.concatenate([res.results[i]["y"] for i in range(N_CORES)], axis=0)
    y_mask = np.ones((B, S), dtype=np.float32)
    return (y, y_mask)
